# revision 14
# baseline (speedup 1.0000x reference)
"""DSA varlen sparse attention for Trainium2, 8 NeuronCores — v2.

Mesh: 8 cores = 4 token-groups x 2 head-groups.  Each core handles 512
tokens x 4 heads against all 2048 keys (dense per-head scores on the PE
array; softmax's Z cancels in the reference's renormalization, so
  out[t,h] = sum_j exp(s[j,t]) w[j,t] V[j,h] / sum_j exp(s[j,t]) w[j,t]
needs no max/Z pass).  Host pre-dedups duplicate topk indices (all slots
of a duplicate group carry the group-summed weight), so there is no
on-device scatter/dedup/transpose machinery at all: the dense weight
masks ship pre-transposed in [j, t] layout.

Per head the 16 key-chunk score groups split two ways to balance ACT/DVE:
 - j-chunks 0-11: ACT exp (scale=1/A) then one DVE multiply by the exact
   bf16 weight mask mW.
 - j-chunks 12-15: a single fused DVE scalar_tensor_tensor per group:
   int16((A*s + B) + lW) bitcast to bf16  ==  exp(s)*w to ~2% (Schraudolph
   in bf16 bit-space; lW = A*ln(w) for selected entries, -46000 masked so
   the bitcast lands on negligible negative denormals).
AV matmuls carry the denominator as a leading ones-column of V; the final
num/den division and unsharding happen host-side.
"""

import numpy as np
import ml_dtypes
from contextlib import ExitStack

T, H, D, DV, TK = 2048, 8, 128, 128, 64
NCORES = 8
NTG, NHG = 4, 2            # token groups x head groups
TC = T // NTG              # 512 tokens per core
NH = H // NHG              # 4 heads per core
P = 128
TCH = TC // P              # 4 token chunks of 128
JC = T // P                # 16 key chunks of 128
NSTT = 6                   # j-chunks drained via fused DVE schraudolph
NACT = JC - NSTT           # j-chunks drained via ACT exp + DVE mask
G = 2                      # j-chunks per PSUM score group
NG = JC // G               # score groups per head
SCALE = float(D) ** -0.5
A = 128.0 / float(np.log(2.0))   # schraudolph scale for bf16 bitcast
B = 16256.0 - 7.5                # (127<<7) minus centering constant

_CACHE = {}


def _build_program():
    import concourse.mybir as mybir
    import concourse.tile as tile
    from concourse import bacc

    dt = mybir.dt
    Alu = mybir.AluOpType
    Act = mybir.ActivationFunctionType

    nc = bacc.Bacc(None, target_bir_lowering=False, debug=False)
    names = {}
    with ExitStack() as ctx:
        tc = ctx.enter_context(tile.TileContext(nc))
        dram = ctx.enter_context(tc.tile_pool(name="dram", bufs=1, space="DRAM"))
        sb = ctx.enter_context(tc.tile_pool(name="sb", bufs=1))
        sps = ctx.enter_context(tc.tile_pool(name="spsum", bufs=2, space="PSUM"))
        ops = ctx.enter_context(tc.tile_pool(name="opsum", bufs=4, space="PSUM"))

        q_d = dram.tile([P, NH * TC], dt.bfloat16, kind="ExternalInput")
        k_d = dram.tile([P, NH * T], dt.bfloat16, kind="ExternalInput")
        v_d = dram.tile([P, NH * JC * (1 + DV)], dt.bfloat16, kind="ExternalInput")
        m_d = dram.tile([P, NACT * TC], dt.bfloat16, kind="ExternalInput")
        l_d = dram.tile([P, NSTT * TC], dt.bfloat16, kind="ExternalInput")
        out_d = dram.tile([P, TCH * NH * (1 + DV)], dt.float32, kind="ExternalOutput")
        names.update(
            q=q_d.name, k=k_d.name, v=v_d.name, m=m_d.name, l=l_d.name,
            out=out_d.name,
        )

        qT = sb.tile([P, NH, TC], dt.bfloat16, tag="qT")        # [d, h, t]
        kT = sb.tile([P, NH, T], dt.bfloat16, tag="kT")         # [d, h, j]
        vE = sb.tile([P, NH, JC, 1 + DV], dt.bfloat16, tag="vE")
        mW = sb.tile([P, TCH, NACT, P], dt.bfloat16, tag="mW")  # [jm, tc, jc, t]
        lW = sb.tile([P, NSTT, TC], dt.bfloat16, tag="lW")
        pT = sb.tile([P, NH, JC, TC], dt.bfloat16, tag="pT")    # weights [j, t]
        pTi = pT.bitcast(dt.int16)
        outs = sb.tile([P, TCH, NH, 1 + DV], dt.float32, tag="outs")

        # ---- PE warm-up: dummy matmuls on garbage SBUF so HAM reaches
        # 2.4 GHz before the first real score matmul ----
        scratch = sb.tile([P, TC], dt.bfloat16, tag="scratch")
        nc.gpsimd.memset(scratch[:], 0.0)
        warm = sps.tile([P, G, TC], dt.float32, tag="sp")
        for i in range(5):
            nc.tensor.matmul(
                out=warm[:, 0, :], lhsT=scratch[:, 0:P], rhs=scratch[:],
                start=True, stop=True,
            )

        # ---- loads (single HWDGE queue: FIFO order == priority;
        # JIT-ordered so the first score/stt consumers unblock early) ----
        nc.sync.dma_start(out=qT[:, 0, :], in_=q_d[:, 0:TC])
        nc.sync.dma_start(out=kT[:, 0, :], in_=k_d[:, 0:T])
        nc.sync.dma_start(out=lW[:].rearrange("p a b -> p (a b)"), in_=l_d[:])
        nc.sync.dma_start(
            out=qT[:, 1:NH, :].rearrange("p a b -> p (a b)"), in_=q_d[:, TC:]
        )
        nc.sync.dma_start(out=kT[:, 1, :], in_=k_d[:, T : 2 * T])
        nc.sync.dma_start(out=kT[:, 2, :], in_=k_d[:, 2 * T : 3 * T])
        MB = NACT * P
        for t in range(TCH):
            nc.sync.dma_start(
                out=mW[:, t].rearrange("p a b -> p (a b)"),
                in_=m_d[:, t * MB : (t + 1) * MB],
            )
        nc.sync.dma_start(out=kT[:, 3, :], in_=k_d[:, 3 * T : 4 * T])
        VB = JC * (1 + DV)
        for h in range(NH):
            nc.sync.dma_start(
                out=vE[:, h].rearrange("p a b -> p (a b)"),
                in_=v_d[:, h * VB : (h + 1) * VB],
            )

        def emit_score_group(h, g):
            sp = sps.tile([P, G, TC], dt.float32, tag="sp")
            for j in range(G):
                jc = g * G + j
                nc.tensor.matmul(
                    out=sp[:, j, :],
                    lhsT=kT[:, h, jc * P : (jc + 1) * P],
                    rhs=qT[:, h, :],
                    start=True, stop=True,
                )
            if g < NACT // G:
                nc.scalar.activation(
                    out=pT[:, h, g * G : (g + 1) * G, :], in_=sp[:],
                    func=Act.Exp, scale=1.0 / A,
                )
            else:
                lg = g - NACT // G
                nc.vector.scalar_tensor_tensor(
                    out=pTi[:, h, g * G : (g + 1) * G, :],
                    in0=sp[:], scalar=float(B),
                    in1=lW[:, lg * G : (lg + 1) * G, :],
                    op0=Alu.add, op1=Alu.add,
                )

        # stt groups (g5-7) alternate with ACT groups so both drain
        # engines run concurrently and the S-phase stays PE-paced
        GORDER = [5, 0, 6, 1, 7, 2, 3, 4]

        def emit_scores(h):
            for g in GORDER:
                emit_score_group(h, g)

        def emit_mask(h):
            # per token-chunk so downstream AVs unblock incrementally and
            # the mask DMA blocks stream in tc-major order
            for t in range(TCH):
                nc.vector.tensor_tensor(
                    out=pT[:, h, 0:NACT, t * P : (t + 1) * P],
                    in0=pT[:, h, 0:NACT, t * P : (t + 1) * P],
                    in1=mW[:, t],
                    op=Alu.mult,
                )

        av_state = {}

        def emit_av(h, t):
            # two 1KB accumulator slots per 2KB PSUM bank-buffer
            n = h * TCH + t
            if n % 2 == 0:
                opt = ops.tile([P, 2, 256], dt.float32, tag="op")
                av_state["tile"] = opt
            op = av_state["tile"][:, n % 2, 0 : 1 + DV]
            for jc in range(JC):
                nc.tensor.matmul(
                    out=op,
                    lhsT=pT[:, h, jc, t * P : (t + 1) * P],
                    rhs=vE[:, h, jc, :],
                    start=(jc == 0), stop=(jc == JC - 1),
                )
            return op

        def emit_drain(h, t, op, act):
            dst = outs[:, t, h, :]
            if act:
                nc.scalar.copy(out=dst, in_=op)
            else:
                nc.vector.tensor_copy(out=dst, in_=op)
            off = (t * NH + h) * (1 + DV)
            nc.sync.dma_start(out=out_d[:, off : off + (1 + DV)], in_=dst)

        emit_scores(0)
        emit_scores(1)
        emit_mask(0)
        emit_scores(2)
        emit_mask(1)
        # interleave the last head's score groups with head 0's AVs so
        # PE stays dense and head-0 drains land early in the ACT queue
        for i, g in enumerate(GORDER):
            emit_score_group(3, g)
            if i % 2 == 1:
                t = i // 2
                op = emit_av(0, t)
                emit_drain(0, t, op, act=True)
        emit_mask(2)
        for t in range(TCH):
            op = emit_av(1, t)
            emit_drain(1, t, op, act=True)
        emit_mask(3)
        for h in (2, 3):
            for t in range(TCH):
                op = emit_av(h, t)
                emit_drain(h, t, op, act=True)

    nc.compile()
    return nc, names


def _get_program():
    if "prog" not in _CACHE:
        _CACHE["prog"] = _build_program()
    return _CACHE["prog"]


def _host_inputs(q, k, v, idx, ts):
    """Per-core in_maps (host-side shard/layout/dtype prep)."""
    bf16 = ml_dtypes.bfloat16
    q = np.asarray(q, np.float32)
    k = np.asarray(k, np.float32)
    v = np.asarray(v, np.float32)
    idx = np.asarray(idx)
    ts = np.asarray(ts, np.float32)

    # Dedup: every slot of a duplicate-index group gets the group sum, so
    # dense-scatter writes (same value) are order-independent.
    eqm = idx[:, :, None] == idx[:, None, :]
    ws = np.einsum("tkl,tl->tk", eqm.astype(np.float32), ts)
    lw = A * np.log(np.maximum(ws, 1e-9))

    Wfull = np.zeros((T, T), np.float32)          # [j, t] mult mask
    Lfull = np.full((T, T), -46000.0, np.float32)  # [j, t] additive log mask
    tt = np.repeat(np.arange(T), TK)
    jj = idx.ravel()
    Wfull[jj, tt] = ws.ravel()
    Lfull[jj, tt] = lw.ravel()

    kT_hg, vE_hg = [], []
    for hg in range(NHG):
        hs = slice(hg * NH, (hg + 1) * NH)
        kk = np.ascontiguousarray(k[:, hs, :].transpose(2, 1, 0)).reshape(
            P, NH * T
        )
        kT_hg.append(kk.astype(bf16))
        vv = v[:, hs, :].reshape(JC, P, NH, DV).transpose(1, 2, 0, 3)
        ve = np.ones((P, NH, JC, 1 + DV), np.float32)
        ve[..., 1:] = vv
        vE_hg.append(ve.reshape(P, NH * JC * (1 + DV)).astype(bf16))

    mW_tg, lW_tg = [], []
    for tg in range(NTG):
        tsl = slice(tg * TC, (tg + 1) * TC)
        # tc-major: [P, TCH, NACT, 128] so each token-chunk block is one
        # contiguous DMA
        Wc = Wfull[: NACT * P, tsl].reshape(NACT, P, TCH, P).transpose(
            1, 2, 0, 3
        )
        mW_tg.append(
            np.ascontiguousarray(Wc).reshape(P, NACT * TC).astype(bf16)
        )
        Lc = Lfull[NACT * P :, tsl].reshape(NSTT, P, TC).transpose(1, 0, 2)
        lW_tg.append(
            np.ascontiguousarray(Lc).reshape(P, NSTT * TC).astype(bf16)
        )

    qs = q * (SCALE * A)
    maps = []
    for c in range(NCORES):
        tg, hg = divmod(c, NHG)
        tsl = slice(tg * TC, (tg + 1) * TC)
        hs = slice(hg * NH, (hg + 1) * NH)
        qc = np.ascontiguousarray(qs[tsl, hs, :].transpose(2, 1, 0)).reshape(
            P, NH * TC
        )
        maps.append(
            dict(q=qc.astype(bf16), k=kT_hg[hg], v=vE_hg[hg], m=mW_tg[tg],
                 l=lW_tg[tg])
        )
    return maps


def kernel(q_packed, k_packed, v_packed, topk_indices, topk_scores):
    from concourse.bass_utils import run_bass_kernel_spmd

    nc, names = _get_program()
    logical = _host_inputs(q_packed, k_packed, v_packed, topk_indices,
                           topk_scores)
    in_maps = [{names[key]: arr for key, arr in m.items()} for m in logical]
    res = run_bass_kernel_spmd(nc, in_maps, core_ids=list(range(NCORES)))

    outn = names["out"]
    out = np.empty((T, H, DV), np.float32)
    for c in range(NCORES):
        tg, hg = divmod(c, NHG)
        oc = np.asarray(res.results[c][outn], np.float32).reshape(
            P, TCH, NH, 1 + DV
        )
        o = oc[..., 1:] / oc[..., 0:1]
        o = o.transpose(1, 0, 2, 3).reshape(TC, NH, DV)
        out[tg * TC : (tg + 1) * TC, hg * NH : (hg + 1) * NH, :] = o
    return out


if __name__ == "__main__":
    rng = np.random.default_rng(0)
    q = rng.standard_normal((T, H, D), dtype=np.float32)
    k = rng.standard_normal((T, H, D), dtype=np.float32)
    v = rng.standard_normal((T, H, DV), dtype=np.float32)
    idx = rng.integers(0, T, size=(T, TK), dtype=np.int64)
    ts = rng.random((T, TK), dtype=np.float32)
    out = kernel(q, k, v, idx, ts)
    print(out.shape, out.dtype)


# revision 15
# speedup vs baseline: 1.0303x; 1.0303x over previous
"""DSA varlen sparse attention for Trainium2, 8 NeuronCores — v2.

Mesh: 8 cores = 4 token-groups x 2 head-groups.  Each core handles 512
tokens x 4 heads against all 2048 keys (dense per-head scores on the PE
array; softmax's Z cancels in the reference's renormalization, so
  out[t,h] = sum_j exp(s[j,t]) w[j,t] V[j,h] / sum_j exp(s[j,t]) w[j,t]
needs no max/Z pass).  Host pre-dedups duplicate topk indices (all slots
of a duplicate group carry the group-summed weight), so there is no
on-device scatter/dedup/transpose machinery at all: the dense weight
masks ship pre-transposed in [j, t] layout.

Per head the 16 key-chunk score groups split two ways to balance ACT/DVE:
 - j-chunks 0-11: ACT exp (scale=1/A) then one DVE multiply by the exact
   bf16 weight mask mW.
 - j-chunks 12-15: a single fused DVE scalar_tensor_tensor per group:
   int16((A*s + B) + lW) bitcast to bf16  ==  exp(s)*w to ~2% (Schraudolph
   in bf16 bit-space; lW = A*ln(w) for selected entries, -46000 masked so
   the bitcast lands on negligible negative denormals).
AV matmuls carry the denominator as a leading ones-column of V; the final
num/den division and unsharding happen host-side.
"""

import numpy as np
import ml_dtypes
from contextlib import ExitStack

T, H, D, DV, TK = 2048, 8, 128, 128, 64
NCORES = 8
NTG, NHG = 4, 2            # token groups x head groups
TC = T // NTG              # 512 tokens per core
NH = H // NHG              # 4 heads per core
P = 128
TCH = TC // P              # 4 token chunks of 128
JC = T // P                # 16 key chunks of 128
NSTT = 6                   # j-chunks drained via fused DVE schraudolph
NACT = JC - NSTT           # j-chunks drained via ACT exp + DVE mask
G = 2                      # j-chunks per PSUM score group
NG = JC // G               # score groups per head
SCALE = float(D) ** -0.5
A = 128.0 / float(np.log(2.0))   # schraudolph scale for bf16 bitcast
B = 16256.0 - 7.5                # (127<<7) minus centering constant

_CACHE = {}


def _build_program():
    import concourse.mybir as mybir
    import concourse.tile as tile
    from concourse import bacc

    dt = mybir.dt
    Alu = mybir.AluOpType
    Act = mybir.ActivationFunctionType

    nc = bacc.Bacc(None, target_bir_lowering=False, debug=False)
    names = {}
    with ExitStack() as ctx:
        tc = ctx.enter_context(tile.TileContext(nc))
        dram = ctx.enter_context(tc.tile_pool(name="dram", bufs=1, space="DRAM"))
        sb = ctx.enter_context(tc.tile_pool(name="sb", bufs=1))
        sps = ctx.enter_context(tc.tile_pool(name="spsum", bufs=3, space="PSUM"))
        ops = ctx.enter_context(tc.tile_pool(name="opsum", bufs=2, space="PSUM"))

        q_d = dram.tile([P, NH * TC], dt.bfloat16, kind="ExternalInput")
        k_d = dram.tile([P, NH * T], dt.bfloat16, kind="ExternalInput")
        v_d = dram.tile([P, NH * JC * (1 + DV)], dt.bfloat16, kind="ExternalInput")
        m_d = dram.tile([P, NACT * TC], dt.bfloat16, kind="ExternalInput")
        l_d = dram.tile([P, NSTT * TC], dt.bfloat16, kind="ExternalInput")
        out_d = dram.tile([P, TCH * NH * (1 + DV)], dt.float32, kind="ExternalOutput")
        names.update(
            q=q_d.name, k=k_d.name, v=v_d.name, m=m_d.name, l=l_d.name,
            out=out_d.name,
        )

        qT = sb.tile([P, NH, TC], dt.bfloat16, tag="qT")        # [d, h, t]
        kT = sb.tile([P, NH, T], dt.bfloat16, tag="kT")         # [d, h, j]
        vE = sb.tile([P, NH, JC, 1 + DV], dt.bfloat16, tag="vE")
        mW = sb.tile([P, TCH, NACT, P], dt.bfloat16, tag="mW")  # [jm, tc, jc, t]
        lW = sb.tile([P, NSTT, TC], dt.bfloat16, tag="lW")
        pT = sb.tile([P, NH, JC, TC], dt.bfloat16, tag="pT")    # weights [j, t]
        pTi = pT.bitcast(dt.int16)
        outs = sb.tile([P, TCH, NH, 1 + DV], dt.float32, tag="outs")

        # ---- PE warm-up: dummy matmuls on garbage SBUF so HAM reaches
        # 2.4 GHz before the first real score matmul ----
        scratch = sb.tile([P, TC], dt.bfloat16, tag="scratch")
        nc.gpsimd.memset(scratch[:], 0.0)
        warm = sps.tile([P, G, TC], dt.float32, tag="sp")
        for i in range(5):
            nc.tensor.matmul(
                out=warm[:, 0, :], lhsT=scratch[:, 0:P], rhs=scratch[:],
                start=True, stop=True,
            )

        # ---- loads (single HWDGE queue: FIFO order == priority;
        # JIT-ordered so the first score/stt consumers unblock early) ----
        nc.sync.dma_start(out=qT[:, 0, :], in_=q_d[:, 0:TC])
        nc.sync.dma_start(out=kT[:, 0, :], in_=k_d[:, 0:T])
        nc.sync.dma_start(out=lW[:].rearrange("p a b -> p (a b)"), in_=l_d[:])
        nc.sync.dma_start(
            out=qT[:, 1:NH, :].rearrange("p a b -> p (a b)"), in_=q_d[:, TC:]
        )
        nc.sync.dma_start(out=kT[:, 1, :], in_=k_d[:, T : 2 * T])
        nc.sync.dma_start(out=kT[:, 2, :], in_=k_d[:, 2 * T : 3 * T])
        MB = NACT * P
        for t in range(TCH):
            nc.sync.dma_start(
                out=mW[:, t].rearrange("p a b -> p (a b)"),
                in_=m_d[:, t * MB : (t + 1) * MB],
            )
        nc.sync.dma_start(out=kT[:, 3, :], in_=k_d[:, 3 * T : 4 * T])
        VB = JC * (1 + DV)
        for h in range(NH):
            nc.sync.dma_start(
                out=vE[:, h].rearrange("p a b -> p (a b)"),
                in_=v_d[:, h * VB : (h + 1) * VB],
            )

        def emit_score_group(h, g):
            sp = sps.tile([P, G, TC], dt.float32, tag="sp")
            for j in range(G):
                jc = g * G + j
                nc.tensor.matmul(
                    out=sp[:, j, :],
                    lhsT=kT[:, h, jc * P : (jc + 1) * P],
                    rhs=qT[:, h, :],
                    start=True, stop=True,
                )
            if g < NACT // G:
                nc.scalar.activation(
                    out=pT[:, h, g * G : (g + 1) * G, :], in_=sp[:],
                    func=Act.Exp, scale=1.0 / A,
                )
            else:
                lg = g - NACT // G
                nc.vector.scalar_tensor_tensor(
                    out=pTi[:, h, g * G : (g + 1) * G, :],
                    in0=sp[:], scalar=float(B),
                    in1=lW[:, lg * G : (lg + 1) * G, :],
                    op0=Alu.add, op1=Alu.add,
                )

        # stt groups (g5-7) alternate with ACT groups so both drain
        # engines run concurrently and the S-phase stays PE-paced
        GORDER = [5, 0, 6, 1, 7, 2, 3, 4]

        def emit_scores(h):
            for g in GORDER:
                emit_score_group(h, g)

        def emit_mask(h):
            # per token-chunk so downstream AVs unblock incrementally and
            # the mask DMA blocks stream in tc-major order
            for t in range(TCH):
                nc.vector.tensor_tensor(
                    out=pT[:, h, 0:NACT, t * P : (t + 1) * P],
                    in0=pT[:, h, 0:NACT, t * P : (t + 1) * P],
                    in1=mW[:, t],
                    op=Alu.mult,
                )

        av_state = {}

        def emit_av(h, t):
            # two 1KB accumulator slots per 2KB PSUM bank-buffer
            n = h * TCH + t
            if n % 2 == 0:
                opt = ops.tile([P, 2, 256], dt.float32, tag="op")
                av_state["tile"] = opt
            op = av_state["tile"][:, n % 2, 0 : 1 + DV]
            for jc in range(JC):
                nc.tensor.matmul(
                    out=op,
                    lhsT=pT[:, h, jc, t * P : (t + 1) * P],
                    rhs=vE[:, h, jc, :],
                    start=(jc == 0), stop=(jc == JC - 1),
                )
            return op

        def emit_drain(h, t, op, act):
            dst = outs[:, t, h, :]
            if act:
                nc.scalar.copy(out=dst, in_=op)
            else:
                nc.vector.tensor_copy(out=dst, in_=op)
            off = (t * NH + h) * (1 + DV)
            nc.sync.dma_start(out=out_d[:, off : off + (1 + DV)], in_=dst)

        emit_scores(0)
        emit_scores(1)
        emit_mask(0)
        emit_scores(2)
        emit_mask(1)
        # interleave the last head's score groups with head 0's AVs so
        # PE stays dense and head-0 drains land early in the ACT queue
        for i, g in enumerate(GORDER):
            emit_score_group(3, g)
            if i % 2 == 1:
                t = i // 2
                op = emit_av(0, t)
                emit_drain(0, t, op, act=True)
        emit_mask(2)
        for t in range(TCH):
            op = emit_av(1, t)
            emit_drain(1, t, op, act=True)
        emit_mask(3)
        for h in (2, 3):
            for t in range(TCH):
                op = emit_av(h, t)
                emit_drain(h, t, op, act=True)

    nc.compile()
    return nc, names


def _get_program():
    if "prog" not in _CACHE:
        _CACHE["prog"] = _build_program()
    return _CACHE["prog"]


def _host_inputs(q, k, v, idx, ts):
    """Per-core in_maps (host-side shard/layout/dtype prep)."""
    bf16 = ml_dtypes.bfloat16
    q = np.asarray(q, np.float32)
    k = np.asarray(k, np.float32)
    v = np.asarray(v, np.float32)
    idx = np.asarray(idx)
    ts = np.asarray(ts, np.float32)

    # Dedup: every slot of a duplicate-index group gets the group sum, so
    # dense-scatter writes (same value) are order-independent.
    eqm = idx[:, :, None] == idx[:, None, :]
    ws = np.einsum("tkl,tl->tk", eqm.astype(np.float32), ts)
    lw = A * np.log(np.maximum(ws, 1e-9))

    Wfull = np.zeros((T, T), np.float32)          # [j, t] mult mask
    Lfull = np.full((T, T), -46000.0, np.float32)  # [j, t] additive log mask
    tt = np.repeat(np.arange(T), TK)
    jj = idx.ravel()
    Wfull[jj, tt] = ws.ravel()
    Lfull[jj, tt] = lw.ravel()

    kT_hg, vE_hg = [], []
    for hg in range(NHG):
        hs = slice(hg * NH, (hg + 1) * NH)
        kk = np.ascontiguousarray(k[:, hs, :].transpose(2, 1, 0)).reshape(
            P, NH * T
        )
        kT_hg.append(kk.astype(bf16))
        vv = v[:, hs, :].reshape(JC, P, NH, DV).transpose(1, 2, 0, 3)
        ve = np.ones((P, NH, JC, 1 + DV), np.float32)
        ve[..., 1:] = vv
        vE_hg.append(ve.reshape(P, NH * JC * (1 + DV)).astype(bf16))

    mW_tg, lW_tg = [], []
    for tg in range(NTG):
        tsl = slice(tg * TC, (tg + 1) * TC)
        # tc-major: [P, TCH, NACT, 128] so each token-chunk block is one
        # contiguous DMA
        Wc = Wfull[: NACT * P, tsl].reshape(NACT, P, TCH, P).transpose(
            1, 2, 0, 3
        )
        mW_tg.append(
            np.ascontiguousarray(Wc).reshape(P, NACT * TC).astype(bf16)
        )
        Lc = Lfull[NACT * P :, tsl].reshape(NSTT, P, TC).transpose(1, 0, 2)
        lW_tg.append(
            np.ascontiguousarray(Lc).reshape(P, NSTT * TC).astype(bf16)
        )

    qs = q * (SCALE * A)
    maps = []
    for c in range(NCORES):
        tg, hg = divmod(c, NHG)
        tsl = slice(tg * TC, (tg + 1) * TC)
        hs = slice(hg * NH, (hg + 1) * NH)
        qc = np.ascontiguousarray(qs[tsl, hs, :].transpose(2, 1, 0)).reshape(
            P, NH * TC
        )
        maps.append(
            dict(q=qc.astype(bf16), k=kT_hg[hg], v=vE_hg[hg], m=mW_tg[tg],
                 l=lW_tg[tg])
        )
    return maps


def kernel(q_packed, k_packed, v_packed, topk_indices, topk_scores):
    from concourse.bass_utils import run_bass_kernel_spmd

    nc, names = _get_program()
    logical = _host_inputs(q_packed, k_packed, v_packed, topk_indices,
                           topk_scores)
    in_maps = [{names[key]: arr for key, arr in m.items()} for m in logical]
    res = run_bass_kernel_spmd(nc, in_maps, core_ids=list(range(NCORES)))

    outn = names["out"]
    out = np.empty((T, H, DV), np.float32)
    for c in range(NCORES):
        tg, hg = divmod(c, NHG)
        oc = np.asarray(res.results[c][outn], np.float32).reshape(
            P, TCH, NH, 1 + DV
        )
        o = oc[..., 1:] / oc[..., 0:1]
        o = o.transpose(1, 0, 2, 3).reshape(TC, NH, DV)
        out[tg * TC : (tg + 1) * TC, hg * NH : (hg + 1) * NH, :] = o
    return out


if __name__ == "__main__":
    rng = np.random.default_rng(0)
    q = rng.standard_normal((T, H, D), dtype=np.float32)
    k = rng.standard_normal((T, H, D), dtype=np.float32)
    v = rng.standard_normal((T, H, DV), dtype=np.float32)
    idx = rng.integers(0, T, size=(T, TK), dtype=np.int64)
    ts = rng.random((T, TK), dtype=np.float32)
    out = kernel(q, k, v, idx, ts)
    print(out.shape, out.dtype)


# revision 16
# speedup vs baseline: 1.0589x; 1.0278x over previous
"""DSA varlen sparse attention for Trainium2, 8 NeuronCores — v2.

Mesh: 8 cores = 4 token-groups x 2 head-groups.  Each core handles 512
tokens x 4 heads against all 2048 keys (dense per-head scores on the PE
array; softmax's Z cancels in the reference's renormalization, so
  out[t,h] = sum_j exp(s[j,t]) w[j,t] V[j,h] / sum_j exp(s[j,t]) w[j,t]
needs no max/Z pass).  Host pre-dedups duplicate topk indices (all slots
of a duplicate group carry the group-summed weight), so there is no
on-device scatter/dedup/transpose machinery at all: the dense weight
masks ship pre-transposed in [j, t] layout.

Per head the 16 key-chunk score groups split two ways to balance ACT/DVE:
 - j-chunks 0-11: ACT exp (scale=1/A) then one DVE multiply by the exact
   bf16 weight mask mW.
 - j-chunks 12-15: a single fused DVE scalar_tensor_tensor per group:
   int16((A*s + B) + lW) bitcast to bf16  ==  exp(s)*w to ~2% (Schraudolph
   in bf16 bit-space; lW = A*ln(w) for selected entries, -46000 masked so
   the bitcast lands on negligible negative denormals).
AV matmuls carry the denominator as a leading ones-column of V; the final
num/den division and unsharding happen host-side.
"""

import numpy as np
import ml_dtypes
from contextlib import ExitStack

T, H, D, DV, TK = 2048, 8, 128, 128, 64
NCORES = 8
NTG, NHG = 4, 2            # token groups x head groups
TC = T // NTG              # 512 tokens per core
NH = H // NHG              # 4 heads per core
P = 128
TCH = TC // P              # 4 token chunks of 128
JC = T // P                # 16 key chunks of 128
NSTT = 6                   # j-chunks drained via fused DVE schraudolph
NACT = JC - NSTT           # j-chunks drained via ACT exp + DVE mask
G = 2                      # j-chunks per PSUM score group
NG = JC // G               # score groups per head
SCALE = float(D) ** -0.5
A = 128.0 / float(np.log(2.0))   # schraudolph scale for bf16 bitcast
B = 16256.0 - 7.5                # (127<<7) minus centering constant

_CACHE = {}


def _build_program():
    import concourse.mybir as mybir
    import concourse.tile as tile
    from concourse import bacc

    dt = mybir.dt
    Alu = mybir.AluOpType
    Act = mybir.ActivationFunctionType

    nc = bacc.Bacc(None, target_bir_lowering=False, debug=False)
    names = {}
    with ExitStack() as ctx:
        tc = ctx.enter_context(tile.TileContext(nc))
        dram = ctx.enter_context(tc.tile_pool(name="dram", bufs=1, space="DRAM"))
        sb = ctx.enter_context(tc.tile_pool(name="sb", bufs=1))
        sps = ctx.enter_context(tc.tile_pool(name="spsum", bufs=3, space="PSUM"))
        ops = ctx.enter_context(tc.tile_pool(name="opsum", bufs=2, space="PSUM"))

        q_d = dram.tile([P, NH * TC], dt.bfloat16, kind="ExternalInput")
        k_d = dram.tile([P, NH * T], dt.bfloat16, kind="ExternalInput")
        v_d = dram.tile([P, NH * JC * (1 + DV)], dt.bfloat16, kind="ExternalInput")
        m_d = dram.tile([P, NACT * TC], dt.bfloat16, kind="ExternalInput")
        l_d = dram.tile([P, NSTT * TC], dt.bfloat16, kind="ExternalInput")
        out_d = dram.tile([P, TCH * NH * (1 + DV)], dt.float32, kind="ExternalOutput")
        names.update(
            q=q_d.name, k=k_d.name, v=v_d.name, m=m_d.name, l=l_d.name,
            out=out_d.name,
        )

        qT = sb.tile([P, NH, TC], dt.bfloat16, tag="qT")        # [d, h, t]
        kT = sb.tile([P, NH, T], dt.bfloat16, tag="kT")         # [d, h, j]
        vE = sb.tile([P, NH, JC, 1 + DV], dt.bfloat16, tag="vE")
        mW = sb.tile([P, TCH, NACT, P], dt.bfloat16, tag="mW")  # [jm, tc, jc, t]
        lW = sb.tile([P, NSTT, TC], dt.bfloat16, tag="lW")
        pT = sb.tile([P, NH, JC, TC], dt.bfloat16, tag="pT")    # weights [j, t]
        pTi = pT.bitcast(dt.int16)
        outs = sb.tile([P, TCH, NH, 1 + DV], dt.float32, tag="outs")

        # ---- PE warm-up: dummy matmuls on garbage SBUF so HAM reaches
        # 2.4 GHz before the first real score matmul ----
        scratch = sb.tile([P, TC], dt.bfloat16, tag="scratch")
        nc.gpsimd.memset(scratch[:], 0.0)
        warm = sps.tile([P, G, TC], dt.float32, tag="sp")
        for i in range(5):
            nc.tensor.matmul(
                out=warm[:, 0, :], lhsT=scratch[:, 0:P], rhs=scratch[:],
                start=True, stop=True,
            )

        # ---- loads (single HWDGE queue: FIFO order == priority;
        # JIT-ordered so the first score/stt consumers unblock early) ----
        nc.sync.dma_start(out=qT[:, 0, :], in_=q_d[:, 0:TC])
        nc.sync.dma_start(out=kT[:, 0, :], in_=k_d[:, 0:T])
        nc.sync.dma_start(out=lW[:].rearrange("p a b -> p (a b)"), in_=l_d[:])
        nc.sync.dma_start(
            out=qT[:, 1:NH, :].rearrange("p a b -> p (a b)"), in_=q_d[:, TC:]
        )
        nc.sync.dma_start(out=kT[:, 1, :], in_=k_d[:, T : 2 * T])
        nc.sync.dma_start(out=kT[:, 2, :], in_=k_d[:, 2 * T : 3 * T])
        MB = NACT * P
        for t in range(TCH):
            nc.sync.dma_start(
                out=mW[:, t].rearrange("p a b -> p (a b)"),
                in_=m_d[:, t * MB : (t + 1) * MB],
            )
        nc.sync.dma_start(out=kT[:, 3, :], in_=k_d[:, 3 * T : 4 * T])
        VB = JC * (1 + DV)
        for h in range(NH):
            nc.sync.dma_start(
                out=vE[:, h].rearrange("p a b -> p (a b)"),
                in_=v_d[:, h * VB : (h + 1) * VB],
            )

        def emit_score_group(h, g):
            sp = sps.tile([P, G, TC], dt.float32, tag="sp")
            for j in range(G):
                jc = g * G + j
                nc.tensor.matmul(
                    out=sp[:, j, :],
                    lhsT=kT[:, h, jc * P : (jc + 1) * P],
                    rhs=qT[:, h, :],
                    start=True, stop=True,
                )
            if g < NACT // G:
                nc.scalar.activation(
                    out=pT[:, h, g * G : (g + 1) * G, :], in_=sp[:],
                    func=Act.Exp, scale=1.0 / A,
                )
            else:
                lg = g - NACT // G
                nc.vector.scalar_tensor_tensor(
                    out=pTi[:, h, g * G : (g + 1) * G, :],
                    in0=sp[:], scalar=float(B),
                    in1=lW[:, lg * G : (lg + 1) * G, :],
                    op0=Alu.add, op1=Alu.add,
                )

        # stt groups (g5-7) alternate with ACT groups so both drain
        # engines run concurrently and the S-phase stays PE-paced
        GORDER = [5, 0, 1, 6, 2, 3, 7, 4]

        def emit_scores(h):
            for g in GORDER:
                emit_score_group(h, g)

        def emit_mask(h):
            # per token-chunk so downstream AVs unblock incrementally and
            # the mask DMA blocks stream in tc-major order
            for t in range(TCH):
                nc.vector.tensor_tensor(
                    out=pT[:, h, 0:NACT, t * P : (t + 1) * P],
                    in0=pT[:, h, 0:NACT, t * P : (t + 1) * P],
                    in1=mW[:, t],
                    op=Alu.mult,
                )

        av_state = {}

        def emit_av(h, t):
            # two 1KB accumulator slots per 2KB PSUM bank-buffer
            n = h * TCH + t
            if n % 2 == 0:
                opt = ops.tile([P, 2, 256], dt.float32, tag="op")
                av_state["tile"] = opt
            op = av_state["tile"][:, n % 2, 0 : 1 + DV]
            for jc in range(JC):
                nc.tensor.matmul(
                    out=op,
                    lhsT=pT[:, h, jc, t * P : (t + 1) * P],
                    rhs=vE[:, h, jc, :],
                    start=(jc == 0), stop=(jc == JC - 1),
                )
            return op

        def emit_drain(h, t, op, act):
            dst = outs[:, t, h, :]
            if act:
                nc.scalar.copy(out=dst, in_=op)
            else:
                nc.vector.tensor_copy(out=dst, in_=op)
            off = (t * NH + h) * (1 + DV)
            nc.sync.dma_start(out=out_d[:, off : off + (1 + DV)], in_=dst)

        emit_scores(0)
        emit_scores(1)
        emit_mask(0)
        emit_scores(2)
        emit_mask(1)
        # interleave the last head's score groups with head 0's AVs so
        # PE stays dense and head-0 drains land early in the ACT queue
        for i, g in enumerate(GORDER):
            emit_score_group(3, g)
            if i % 2 == 1:
                t = i // 2
                op = emit_av(0, t)
                emit_drain(0, t, op, act=True)
        emit_mask(2)
        for t in range(TCH):
            op = emit_av(1, t)
            emit_drain(1, t, op, act=True)
        emit_mask(3)
        for h in (2, 3):
            for t in range(TCH):
                op = emit_av(h, t)
                emit_drain(h, t, op, act=True)

    nc.compile()
    return nc, names


def _get_program():
    if "prog" not in _CACHE:
        _CACHE["prog"] = _build_program()
    return _CACHE["prog"]


def _host_inputs(q, k, v, idx, ts):
    """Per-core in_maps (host-side shard/layout/dtype prep)."""
    bf16 = ml_dtypes.bfloat16
    q = np.asarray(q, np.float32)
    k = np.asarray(k, np.float32)
    v = np.asarray(v, np.float32)
    idx = np.asarray(idx)
    ts = np.asarray(ts, np.float32)

    # Dedup: every slot of a duplicate-index group gets the group sum, so
    # dense-scatter writes (same value) are order-independent.
    eqm = idx[:, :, None] == idx[:, None, :]
    ws = np.einsum("tkl,tl->tk", eqm.astype(np.float32), ts)
    lw = A * np.log(np.maximum(ws, 1e-9))

    Wfull = np.zeros((T, T), np.float32)          # [j, t] mult mask
    Lfull = np.full((T, T), -46000.0, np.float32)  # [j, t] additive log mask
    tt = np.repeat(np.arange(T), TK)
    jj = idx.ravel()
    Wfull[jj, tt] = ws.ravel()
    Lfull[jj, tt] = lw.ravel()

    kT_hg, vE_hg = [], []
    for hg in range(NHG):
        hs = slice(hg * NH, (hg + 1) * NH)
        kk = np.ascontiguousarray(k[:, hs, :].transpose(2, 1, 0)).reshape(
            P, NH * T
        )
        kT_hg.append(kk.astype(bf16))
        vv = v[:, hs, :].reshape(JC, P, NH, DV).transpose(1, 2, 0, 3)
        ve = np.ones((P, NH, JC, 1 + DV), np.float32)
        ve[..., 1:] = vv
        vE_hg.append(ve.reshape(P, NH * JC * (1 + DV)).astype(bf16))

    mW_tg, lW_tg = [], []
    for tg in range(NTG):
        tsl = slice(tg * TC, (tg + 1) * TC)
        # tc-major: [P, TCH, NACT, 128] so each token-chunk block is one
        # contiguous DMA
        Wc = Wfull[: NACT * P, tsl].reshape(NACT, P, TCH, P).transpose(
            1, 2, 0, 3
        )
        mW_tg.append(
            np.ascontiguousarray(Wc).reshape(P, NACT * TC).astype(bf16)
        )
        Lc = Lfull[NACT * P :, tsl].reshape(NSTT, P, TC).transpose(1, 0, 2)
        lW_tg.append(
            np.ascontiguousarray(Lc).reshape(P, NSTT * TC).astype(bf16)
        )

    qs = q * (SCALE * A)
    maps = []
    for c in range(NCORES):
        tg, hg = divmod(c, NHG)
        tsl = slice(tg * TC, (tg + 1) * TC)
        hs = slice(hg * NH, (hg + 1) * NH)
        qc = np.ascontiguousarray(qs[tsl, hs, :].transpose(2, 1, 0)).reshape(
            P, NH * TC
        )
        maps.append(
            dict(q=qc.astype(bf16), k=kT_hg[hg], v=vE_hg[hg], m=mW_tg[tg],
                 l=lW_tg[tg])
        )
    return maps


def kernel(q_packed, k_packed, v_packed, topk_indices, topk_scores):
    from concourse.bass_utils import run_bass_kernel_spmd

    nc, names = _get_program()
    logical = _host_inputs(q_packed, k_packed, v_packed, topk_indices,
                           topk_scores)
    in_maps = [{names[key]: arr for key, arr in m.items()} for m in logical]
    res = run_bass_kernel_spmd(nc, in_maps, core_ids=list(range(NCORES)))

    outn = names["out"]
    out = np.empty((T, H, DV), np.float32)
    for c in range(NCORES):
        tg, hg = divmod(c, NHG)
        oc = np.asarray(res.results[c][outn], np.float32).reshape(
            P, TCH, NH, 1 + DV
        )
        o = oc[..., 1:] / oc[..., 0:1]
        o = o.transpose(1, 0, 2, 3).reshape(TC, NH, DV)
        out[tg * TC : (tg + 1) * TC, hg * NH : (hg + 1) * NH, :] = o
    return out


if __name__ == "__main__":
    rng = np.random.default_rng(0)
    q = rng.standard_normal((T, H, D), dtype=np.float32)
    k = rng.standard_normal((T, H, D), dtype=np.float32)
    v = rng.standard_normal((T, H, DV), dtype=np.float32)
    idx = rng.integers(0, T, size=(T, TK), dtype=np.int64)
    ts = rng.random((T, TK), dtype=np.float32)
    out = kernel(q, k, v, idx, ts)
    print(out.shape, out.dtype)
